# revision 15
# baseline (speedup 1.0000x reference)
"""Cross-attention block kernel for Trainium2 (8 NeuronCores, data-parallel).

Computes, for full inputs:
    Q = x @ Wq + bq            [B, HW, D]
    K = a @ Wk + bk            [B, S, D]
    V = a @ Wv + bv            [B, S, D]
    out = softmax(Q K^T / sqrt(D)) @ V

Sharding: batch (B=16) split across 8 cores, 2 batches per core. Weights
replicated. No collectives needed.

Per-core strategy (fp8 e4m3 DoubleRow for the attention contractions):
  - Host pre-work: x/audio/weights cast to bf16; Wq,Wk,Wv,bk,bv scaled by 4
    so the fp8 operands use the e4m3 normal range.
  - x and audio transposed to d-major via the DMA XBAR (dma_start_transpose,
    16x128 tiles) -- the PE does no transposes at all.
  - Projections run in bf16 (1 cycle/row, full PE rate): qT = Wq^T @ xT,
    kT = Wk^T @ aT, v = aT^T @ Wv (+bv via a K=1 ones-row matmul folded into
    the same PSUM group).
  - Q-bias folding: softmax(Q K^T) is invariant to per-query shifts, so
    Q stays raw and the per-key correction delta[s] = bq . K'[s] (K' = K+bk)
    is added via the ACT bias port at the exp stage. delta comes from tiny
    DoubleRow matmuls (ap_size=1) against an e4m3 copy of bq.
  - scores^T[s, hw] = kh^T qh in PLAIN e4m3 DoubleRow (2 k-tiles per
    instruction, 0.5 cycles/row -> 4x fp32r rate). Q/K quantization error is
    damped by the softmax scale (std of scaled scores ~0.33), total ~1%.
  - ex = exp(scale*scores + scale*delta + ln8) computed by ACT straight from
    PSUM, split into an e4m3 hi/lo pair (exh = DVE quantize, exl = DVE sub).
    The ln8 bias scales ex by 8 so the whole range [1.3, 48] is normal e4m3.
  - out = (exh+exl) @ (vh+vl) with the lo*lo term dropped: hi*hi uses
    k-tile-paired DoubleRow; the cross terms exl*vh and exh*vl share single
    DoubleRow instructions (slot0=lo*hi, slot1=hi*lo). 1.33x fp32r rate.
  - denominator: DoubleRow against a constant 4.0 column (matching the 4x
    pre-scale of V), interleaved into a second PSUM bank; the final ACT copy
    applies the reciprocal as a per-partition scale.
  - The out stage is software-pipelined one block behind scores/exp so the
    ACT/DVE queues never stall the PE.
"""

from contextlib import ExitStack

import ml_dtypes
import numpy as np

import concourse.bass as bass
import concourse.bacc as bacc
import concourse.mybir as mybir
import concourse.tile as tile
from concourse.bass_utils import run_bass_kernel_spmd

P = 128
D = 512          # d_query == d_audio == d_out
CD = D // P      # 4 chunks of the feature dim
HW = 4096        # queries per batch
S = 1024         # keys per batch
SC = S // P      # 8 s-chunks
HWB = 512        # hw rows processed per block
NBLK = HW // HWB
B_FULL = 16
N_CORES = 8
BL = B_FULL // N_CORES  # 2 batches per core
SCALE = 1.0 / float(np.sqrt(D))
LN8 = float(np.log(8.0))

f32 = mybir.dt.float32
bf16 = mybir.dt.bfloat16
e4 = mybir.dt.float8e4
AFT = mybir.ActivationFunctionType
ALU = mybir.AluOpType
DR = mybir.MatmulPerfMode.DoubleRow

BF16NP = ml_dtypes.bfloat16


def build_nc():
    nc = bacc.Bacc("TRN2", target_bir_lowering=False, debug=False)

    x = nc.dram_tensor("x", [BL, HW, D], bf16, kind="ExternalInput").ap()
    audio = nc.dram_tensor("audio_embed", [BL, S, D], bf16, kind="ExternalInput").ap()
    wq = nc.dram_tensor("Wq", [D, D], bf16, kind="ExternalInput").ap()
    bq = nc.dram_tensor("bq", [D], f32, kind="ExternalInput").ap()
    wk = nc.dram_tensor("Wk", [D, D], bf16, kind="ExternalInput").ap()
    bk = nc.dram_tensor("bk", [D], f32, kind="ExternalInput").ap()
    wv = nc.dram_tensor("Wv", [D, D], bf16, kind="ExternalInput").ap()
    bv = nc.dram_tensor("bv", [D], bf16, kind="ExternalInput").ap()
    out = nc.dram_tensor("out", [BL, HW, D], bf16, kind="ExternalOutput").ap()

    with tile.TileContext(nc) as tc:
        with ExitStack() as ctx:
            _body(ctx, tc, x, audio, wq, bq, wk, bk, wv, bv, out)

    nc.compile()
    return nc


def _body(ctx, tc, x, audio, wq, bq, wk, bk, wv, bv, out):
    nc = tc.nc

    const_pool = ctx.enter_context(tc.tile_pool(name="const", bufs=1))
    batch_pool = ctx.enter_context(tc.tile_pool(name="batch", bufs=2))
    work_pool = ctx.enter_context(tc.tile_pool(name="work", bufs=2))
    small_pool = ctx.enter_context(tc.tile_pool(name="small", bufs=4))
    psum_mm = ctx.enter_context(tc.tile_pool(name="pmm", bufs=4, space="PSUM"))
    psum_sc = ctx.enter_context(tc.tile_pool(name="psc", bufs=2, space="PSUM"))
    psum_den = ctx.enter_context(tc.tile_pool(name="pden", bufs=1, space="PSUM"))
    psum_dl = ctx.enter_context(tc.tile_pool(name="pdl", bufs=1, space="PSUM"))

    # Weight/bias loads are interleaved with the first audio chunks so the
    # first transposable input data leads the serial DMA queue.
    consts = {}

    def _load_small_consts():
        bk_sb = const_pool.tile([P, CD], f32)
        nc.sync.dma_start(bk_sb, bk.rearrange("(c p) -> p c", p=P))
        bq_f = const_pool.tile([P, CD], f32)
        nc.sync.dma_start(bq_f, bq.rearrange("(c p) -> p c", p=P))
        bq8 = const_pool.tile([P, CD], e4)
        nc.vector.tensor_copy(bq8, bq_f)
        bv_row = const_pool.tile([1, D], bf16)
        nc.sync.dma_start(bv_row, bv[None, :])
        ones_row = const_pool.tile([1, P], bf16)
        nc.gpsimd.memset(ones_row, 1.0)
        fours = const_pool.tile([P, 2, 1], e4)
        nc.gpsimd.memset(fours, 4.0)
        consts.update(bk_sb=bk_sb, bq8=bq8, bv_row=bv_row,
                      ones_row=ones_row, fours=fours)

    def _load_w(name, t, queue=None):
        w_sb = const_pool.tile([P, CD, D], bf16, name=f"w_sb_{name}")
        (queue or nc.sync).dma_start(w_sb, t.rearrange("(c p) n -> p c n", p=P))
        consts[name] = w_sb

    def emit_audio_loads(b):
        """audio DMA + one merged XBAR per half. aT layout [P, half, ch, dc,
        s_p]: logical d = dc*128 + p, s = half*512 + ch*128 + s_p."""
        aT = batch_pool.tile([P, 2, CD, CD, P], bf16, tag="aT")
        a_sb = work_pool.tile([P, 2, CD, D], bf16, tag="a")
        for half in range(2):
            a_view = audio[b].rearrange("(t c p) n -> t p c n", p=P, c=CD)[half]
            if b == 0 and half == 0:
                # chunked first half: each per-chunk XBAR becomes ready (and
                # can grab the DMA engines) ~4x sooner than a whole-half one
                for c in range(CD):
                    nc.sync.dma_start(a_sb[:, 0, c, :], a_view[:, c, :])
                    nc.sync.dma_start_transpose(
                        aT[:, 0, c, :, :], a_sb[:, 0, c, :]
                    )
                _load_w("wk_sb", wk)
            else:
                nc.sync.dma_start(a_sb[:, half], a_view)
                nc.sync.dma_start_transpose(aT[:, half], a_sb[:, half])
                if b == 0 and half == 1:
                    _load_w("wv_sb", wv)
        if b == 0:
            _load_small_consts()
        return aT

    def emit_audio_compute(b, aT):
        """K-hi, V hi/lo, delta for batch b from the transposed audio."""
        kh = batch_pool.tile([P, CD, S], e4, tag="kh")
        vhl = batch_pool.tile([P, 2, SC, D], e4, tag="v")  # [:,0]=hi [:,1]=lo
        dT_ps = psum_dl.tile([P, SC], f32, tag="dl")
        dsb = batch_pool.tile([P, SC], f32, tag="dsb")
        for half in range(2):
            hsl = slice(half * 512, (half + 1) * 512)
            for m in range(CD):
                mm_ps = psum_mm.tile([P, 512], f32, tag="mm")
                for c in range(CD):
                    nc.tensor.matmul(
                        mm_ps,
                        consts["wk_sb"][:, c, m * P : (m + 1) * P],
                        aT[:, half, :, c, :],
                        start=(c == 0),
                        stop=(c == CD - 1),
                    )
                nc.scalar.activation(
                    kh[:, m, hsl], mm_ps, AFT.Identity,
                    bias=consts["bk_sb"][:, m, None], scale=1.0,
                )
            for g in range(half * 4, half * 4 + 4):
                mm_ps = psum_mm.tile([P, D], f32, tag="mm")
                for c in range(CD):
                    nc.tensor.matmul(
                        mm_ps,
                        aT[:, half, g % 4, c, :],
                        consts["wv_sb"][:, c, :],
                        start=(c == 0),
                        stop=False,
                    )
                nc.tensor.matmul(
                    mm_ps, consts["ones_row"], consts["bv_row"],
                    start=False, stop=True,
                )
                nc.scalar.activation(vhl[:, 0, g, :], mm_ps, AFT.Copy)
                nc.vector.tensor_tensor(
                    vhl[:, 1, g, :], mm_ps, vhl[:, 0, g, :], ALU.subtract
                )
            # delta[s] = bq . K'[s] for this half's s-chunks (tiny DoubleRow)
            for g in range(half * 4, half * 4 + 4):
                for t in range(2):
                    nc.tensor.matmul(
                        dT_ps[:, g : g + 1],
                        kh[:, 2 * t : 2 * t + 2, g * P : (g + 1) * P],
                        consts["bq8"][:, 2 * t : 2 * t + 2, None],
                        start=(t == 0),
                        stop=(t == 1),
                        perf_mode=DR,
                    )
        # dsb = (SCALE/4) * dT + ln(8): exp-stage per-partition bias
        nc.vector.tensor_scalar(dsb, dT_ps, SCALE / 4.0, LN8, ALU.mult, ALU.add)
        return {"kh": kh, "vhl": vhl, "dsb": dsb}

    def emit_x_loads(b, blk):
        """x load + merged XBAR -> xT [P, ch, dc, hw_p]: d = dc*128 + p,
        hw = ch*128 + hw_p."""
        x_sb = work_pool.tile([P, CD, D], bf16, tag="x", bufs=4)
        nc.sync.dma_start(
            x_sb, x[b].rearrange("(t c p) n -> t p c n", p=P, c=CD)[blk]
        )
        xT = work_pool.tile([P, CD, CD, P], bf16, tag="xT", bufs=4)
        nc.sync.dma_start_transpose(xT, x_sb)
        return xT

    def emit_q_stage(b, blk, xT, st):
        """q projection (raw, no bias) -> qh e4m3."""
        qh = work_pool.tile([P, CD, HWB], e4, tag="qh")
        for m in range(CD):
            mm_ps = psum_mm.tile([P, HWB], f32, tag="mm")
            for c in range(CD):
                nc.tensor.matmul(
                    mm_ps,
                    consts["wq_sb"][:, c, m * P : (m + 1) * P],
                    xT[:, :, c, :],
                    start=(c == 0),
                    stop=(c == CD - 1),
                )
            nc.vector.tensor_copy(qh[:, m, :], mm_ps)
        st["qh"] = qh

    def emit_scores_stage(bst, st):
        kh, dsb = bst["kh"], bst["dsb"]
        qh = st.pop("qh")
        exlh = work_pool.tile([P, 2, SC, HWB], e4, tag="ex")  # [:,0]=lo [:,1]=hi
        for g in range(SC):
            sc_ps = psum_sc.tile([P, HWB], f32, tag="sc")
            for t in range(2):
                nc.tensor.matmul(
                    sc_ps,
                    kh[:, 2 * t : 2 * t + 2, g * P : (g + 1) * P],
                    qh[:, 2 * t : 2 * t + 2, :],
                    start=(t == 0),
                    stop=(t == 1),
                    perf_mode=DR,
                )
            ex_f = small_pool.tile([P, HWB], f32, tag="exf")
            nc.scalar.activation(
                ex_f, sc_ps, AFT.Exp, bias=dsb[:, g, None], scale=SCALE / 16.0
            )
            nc.vector.tensor_copy(exlh[:, 1, g, :], ex_f)
            nc.vector.tensor_tensor(
                exlh[:, 0, g, :], ex_f, exlh[:, 1, g, :], ALU.subtract
            )
        st["exlh"] = exlh

    def emit_out_stage(bst, st, b, blk, last=False):
        exlh = st.pop("exlh")
        vhl = bst["vhl"]
        out_view = out[b].rearrange("(t h p) n -> t p h n", p=P, h=CD)[blk]
        out_sb = work_pool.tile([P, CD, D], bf16, tag="o")
        den_all = psum_den.tile([P, CD], f32, tag="den")
        for h in range(CD):
            hs = slice(h * P, (h + 1) * P)
            num_ps = psum_mm.tile([P, D], f32, tag="mm")
            den_ps = den_all[:, h : h + 1]
            # hi*hi over k-tile pairs
            for t in range(SC // 2):
                nc.tensor.matmul(
                    num_ps,
                    exlh[:, 1, 2 * t : 2 * t + 2, hs],
                    vhl[:, 0, 2 * t : 2 * t + 2, :],
                    start=(t == 0),
                    stop=False,
                    perf_mode=DR,
                )
            # cross terms: slot0 = exl*vh, slot1 = exh*vl; den interleaved
            for t in range(SC):
                nc.tensor.matmul(
                    num_ps,
                    exlh[:, :, t, hs],
                    vhl[:, :, t, :],
                    start=False,
                    stop=(t == SC - 1),
                    perf_mode=DR,
                )
                nc.tensor.matmul(
                    den_ps,
                    exlh[:, :, t, hs],
                    consts["fours"],
                    start=(t == 0),
                    stop=(t == SC - 1),
                    perf_mode=DR,
                )
            rec = small_pool.tile([P, 1], f32, tag="rec")
            nc.vector.reciprocal(rec, den_ps)
            nc.scalar.activation(out_sb[:, h, :], num_ps, AFT.Copy, bias=0.0, scale=rec)
            if last:
                nc.scalar.dma_start(out_view[:, h, :], out_sb[:, h, :])
        # store on the ACT hwdge queue (so x loads/XBARs never queue behind it)
        if not last:
            nc.scalar.dma_start(out_view, out_sb)

    # --- staged global loop: x loads LEAD steps ahead, qT one block ahead
    # of scores, out one block behind ------------------------------------
    TOT = BL * NBLK
    LEAD = 2
    AUDIO_TRIGGER = 5  # prefetch batch b+1's audio loads at blk 5 of batch b
    bstates = {}
    stages = {}
    aT_pend = {}
    xT_pend = {}
    for s in range(TOT + 2):
        if s < TOT:
            b, blk = divmod(s, NBLK)
            if blk == 0:
                if b == 0:
                    aT_pend[0] = emit_audio_loads(0)
                bstates[b] = emit_audio_compute(b, aT_pend.pop(b))
                if b == 0:
                    xT_pend[0] = emit_x_loads(0, 0)
                    _load_w("wq_sb", wq)
                    for k in range(1, LEAD + 1):
                        xT_pend[k] = emit_x_loads(*divmod(k, NBLK))
            if s + LEAD + 1 < TOT:
                xT_pend[s + LEAD + 1] = emit_x_loads(*divmod(s + LEAD + 1, NBLK))
            st = stages[s] = {}
            emit_q_stage(b, blk, xT_pend.pop(s), st)
            if blk == AUDIO_TRIGGER and b + 1 < BL:
                aT_pend[b + 1] = emit_audio_loads(b + 1)
        if 1 <= s <= TOT:
            b, blk = divmod(s - 1, NBLK)
            emit_scores_stage(bstates[b], stages[s - 1])
        if 2 <= s <= TOT + 1:
            b, blk = divmod(s - 2, NBLK)
            emit_out_stage(bstates[b], stages.pop(s - 2), b, blk, last=(s == TOT + 1))


_NC_CACHE = None


def _get_nc():
    global _NC_CACHE
    if _NC_CACHE is None:
        _NC_CACHE = build_nc()
    return _NC_CACHE


def make_in_maps(inputs):
    """Host-side prep: bf16 casts + 4x scaling of W/bk/bv, per-core slices."""
    x = np.asarray(inputs["x"], dtype=np.float32)
    audio = np.asarray(inputs["audio_embed"], dtype=np.float32)
    wq = (np.asarray(inputs["Wq"], dtype=np.float32) * 4.0).astype(BF16NP)
    bq = np.ascontiguousarray(np.asarray(inputs["bq"], dtype=np.float32))
    wk = (np.asarray(inputs["Wk"], dtype=np.float32) * 4.0).astype(BF16NP)
    bk = np.ascontiguousarray(np.asarray(inputs["bk"], dtype=np.float32) * 4.0)
    wv = (np.asarray(inputs["Wv"], dtype=np.float32) * 4.0).astype(BF16NP)
    bv = (np.asarray(inputs["bv"], dtype=np.float32) * 4.0).astype(BF16NP)
    xb = x.astype(BF16NP)
    ab = audio.astype(BF16NP)
    in_maps = []
    for i in range(N_CORES):
        in_maps.append(
            {
                "x": np.ascontiguousarray(xb[i * BL : (i + 1) * BL]),
                "audio_embed": np.ascontiguousarray(ab[i * BL : (i + 1) * BL]),
                "Wq": wq,
                "bq": bq,
                "Wk": wk,
                "bk": bk,
                "Wv": wv,
                "bv": bv,
            }
        )
    return in_maps


def kernel(**inputs):
    nc = _get_nc()
    in_maps = make_in_maps(inputs)
    res = run_bass_kernel_spmd(nc, in_maps, core_ids=list(range(N_CORES)))
    return np.concatenate(
        [np.asarray(res.results[i]["out"]) for i in range(N_CORES)], axis=0
    ).astype(np.float32)
